# revision 3
# baseline (speedup 1.0000x reference)
"""BitLinear (ternary weight + per-token int8 absmax activation) on 8 trn2 cores.

y = (round(x/s) clipped) * s  @  (alpha * clip(round(W/alpha),-1,1)).T  + bias
  with s = max(absmax(x, -1), eps)/127 per token, alpha = max(mean|W|, eps).

Strategy: data-parallel over tokens (4096 tokens/core).  Weight prep is
sharded: each core ternarizes its 256-row slice of W (host-sliced input),
alpha partials are AllReduced, and the transposed ternary weight (fp8e4)
is AllGathered so every core holds the full W^T for its token matmuls.

The matmul runs in fp8 DoubleRow perf mode (2 moving rows/cycle -- 2x the
bf16 rate).  Integer activations q in [-127,127] are split exactly as
q = hi16 + lo with hi16 = 16-grid round (multiples of 16, |hi16| <= 128)
and lo in [-8,8]; both parts are exact in fp8e4m3, ternary weights
{-1,0,1} likewise, and the PE accumulates in fp32, so the matmul integer
part remains EXACT.  Per-token scale (s*alpha) is applied to the PSUM
result, bias added, all in fp32.  Rounding uses the magic-number trick:
(v + 1.5*2^23 - 1.5*2^23) = IEEE RNE == jnp.round; the 16-grid round uses
1.5*2^27 the same way.  The split survives either ALU intermediate
behavior (fused or rounded) -- both yield an exact fp8-representable
decomposition of q.
"""

import numpy as np
from contextlib import ExitStack

import concourse.bass as bass
from concourse import bacc
import concourse.mybir as mybir
import concourse.tile as tile
from concourse.bass import ts
from concourse.bass_utils import run_bass_kernel_spmd
from concourse.masks import make_identity

P = 128
D_IN = 2048
D_OUT = 2048
KC = D_IN // P          # 16 contraction chunks
NFREE = 512             # matmul free dim (one PSUM bank of f32)
NT = D_OUT // NFREE     # 4 n-chunks
M1 = 12582912.0         # 1.5 * 2**23 : fp32 RNE round-to-int offset
M2 = 201326592.0        # 1.5 * 2**27 : fp32 RNE round-to-16-grid offset
EPS = 1e-5
CLAMP = float(np.nextafter(np.float32(1.5), np.float32(0.0)))  # largest f32 < 1.5
N_CORES = 8
WS_ROWS = D_OUT // N_CORES          # 256 weight rows per core
WS_CH = WS_ROWS // P                # 2 chunks of 128 rows per core

F32 = mybir.dt.float32
BF16 = mybir.dt.bfloat16
FP8 = mybir.dt.float8e4
DR = mybir.MatmulPerfMode.DoubleRow
Copy = mybir.ActivationFunctionType.Copy
Alu = mybir.AluOpType
AX = mybir.AxisListType
GROUPS = [list(range(N_CORES))]


def _build(T: int, repeat: int = 1) -> bass.Bass:
    """Build the per-core program for T tokens (repeat>1: perf timing only)."""
    MS = T // P  # token tiles
    nc = bacc.Bacc(None, target_bir_lowering=False)

    x_d = nc.dram_tensor("x", [T, D_IN], F32, kind="ExternalInput")
    ws_d = nc.dram_tensor("ws", [WS_ROWS, D_IN], F32, kind="ExternalInput")
    b_d = nc.dram_tensor("b", [D_OUT], F32, kind="ExternalInput")
    y_d = nc.dram_tensor("y", [T, D_OUT], F32, kind="ExternalOutput")
    x_v = x_d.rearrange("(s p) d -> s p d", p=P)
    y_v = y_d.rearrange("(s p) d -> s p d", p=P)

    with tile.TileContext(nc) as tc, ExitStack() as ctx:
      const = ctx.enter_context(tc.tile_pool(name="const", bufs=1))
      wload = ctx.enter_context(tc.tile_pool(name="wload", bufs=1))
      wtmp = ctx.enter_context(tc.tile_pool(name="wtmp", bufs=2))
      xin = ctx.enter_context(tc.tile_pool(name="xin", bufs=2))
      xu = ctx.enter_context(tc.tile_pool(name="xu", bufs=2))
      xq = ctx.enter_context(tc.tile_pool(name="xq", bufs=2))
      xt = ctx.enter_context(tc.tile_pool(name="xt", bufs=2))
      xe = ctx.enter_context(tc.tile_pool(name="xe", bufs=2))
      scl = ctx.enter_context(tc.tile_pool(name="scl", bufs=4))
      yout = ctx.enter_context(tc.tile_pool(name="yout", bufs=2))
      psum = ctx.enter_context(tc.tile_pool(name="psum", bufs=2, space="PSUM"))
      dram = ctx.enter_context(tc.tile_pool(name="dram", bufs=1, space="DRAM"))
      for _rep in range(repeat):
        wT = const.tile([P, KC, D_OUT], FP8)           # full ternary W^T, fp8
        bias_bc = const.tile([P, D_OUT], F32)
        ident = const.tile([P, P], F32)
        partial = const.tile([P, WS_CH], F32)
        my_psum = const.tile([P, 1], F32)
        wsum = const.tile([P, 1], F32)
        alpha_sb = const.tile([P, 1], F32)
        inv_alpha = const.tile([P, 1], F32)
        alpha127 = const.tile([P, 1], F32)

        nc.gpsimd.dma_start(out=bias_bc[:], in_=b_d[None, :].to_broadcast((P, D_OUT)))
        make_identity(nc, ident[:])

        # ---- phase W-A: alpha = max(mean|W|, eps), sharded + AllReduce --
        # accuracy matters: the ternary decision boundary sits ~4e-7
        # (relative) from the nearest weight, so sums are grouped small and
        # finished with an explicit pairwise tree (stays ~1 ulp of f64).
        wcs = []
        for c in range(WS_CH):
            wc = wload.tile([P, D_IN], F32, tag=f"wchunk{c}", bufs=1)
            nc.sync.dma_start(out=wc[:], in_=ws_d[ts(c, P), :])
            s1 = scl.tile([P, KC], F32, tag="s1")
            nc.vector.tensor_reduce(
                s1[:], wc.rearrange("p (a b) -> p a b", a=KC), axis=AX.X,
                op=Alu.add, apply_absolute_value=True,
            )
            nc.vector.tensor_reduce(
                partial[:, c : c + 1], s1[:], axis=AX.X, op=Alu.add
            )
            wcs.append(wc)
        nc.vector.tensor_reduce(my_psum[:], partial[:], axis=AX.X, op=Alu.add)
        # AllReduce the per-partition partial sums across cores
        ar_in = dram.tile([P, 1], F32, name="ar_in")
        ar_out = dram.tile([P, 1], F32, name="ar_out", addr_space="Shared")
        nc.sync.dma_start(out=ar_in[:], in_=my_psum[:])
        nc.gpsimd.collective_compute(
            "AllReduce", Alu.add, replica_groups=GROUPS,
            ins=[ar_in[:]], outs=[ar_out[:]],
        )
        nc.sync.dma_start(out=wsum[:], in_=ar_out[:])
        # 128 per-partition totals -> one row (exact PE transpose), then a
        # pairwise tree of 7 adds.
        ps_t = psum.tile([1, P], F32, tag="mm", name="ps_t", bufs=2)
        nc.tensor.transpose(ps_t[:], wsum[:], ident[:])
        row = const.tile([1, P], F32)
        nc.scalar.copy(row[:], ps_t[:])
        width = P // 2
        while width >= 1:
            nc.vector.tensor_tensor(
                row[0:1, 0:width], row[0:1, 0:width],
                row[0:1, width : 2 * width], op=Alu.add,
            )
            width //= 2
        al_sc = const.tile([1, 1], F32)
        nc.vector.tensor_scalar(
            al_sc[:], row[0:1, 0:1], 1.0 / (D_IN * D_OUT), EPS,
            op0=Alu.mult, op1=Alu.max,
        )
        # broadcast alpha to all partitions through a DRAM bounce
        al_d = dram.tile([1, 1], F32, name="al_d")
        nc.sync.dma_start(out=al_d[:], in_=al_sc[:])
        nc.gpsimd.dma_start(out=alpha_sb[:], in_=al_d[:].to_broadcast((P, 1)))
        nc.vector.reciprocal(inv_alpha[:], alpha_sb[:])
        nc.scalar.mul(alpha127[:], alpha_sb[:], 1.0 / 127.0)

        # ---- phase W-B: ternarize own shard + transpose + AllGather -----
        contrib = dram.tile([P, KC, WS_ROWS], FP8, name="contrib")
        gathered = dram.tile([N_CORES, P, KC, WS_ROWS], FP8, name="gathered",
                             addr_space="Shared")
        for c in range(WS_CH):
            nc.scalar.activation(wcs[c][:], wcs[c][:], Copy, scale=inv_alpha[:])
            # clamp to (-1.5, 1.5) so round gives {-1,0,1} (== clip(round,-1,1))
            nc.gpsimd.tensor_scalar(
                wcs[c][:], wcs[c][:], CLAMP, -CLAMP, op0=Alu.min, op1=Alu.max
            )
            wt = wtmp.tile([P, D_IN], BF16, tag="wtern")
            nc.gpsimd.tensor_scalar(
                wt[:], wcs[c][:], M1, M1, op0=Alu.add, op1=Alu.subtract
            )
            wtl = wtmp.tile([P, KC, P], BF16, tag="wtl", bufs=2)
            nc.scalar.dma_start_transpose(wtl[:], wt[:])
            wtl8 = wtmp.tile([P, KC, P], FP8, tag="wtl8", bufs=2)
            nc.vector.tensor_scalar(wtl8[:], wtl[:], 0.0, None, op0=Alu.add)
            nc.sync.dma_start(out=contrib[:, :, ts(c, P)], in_=wtl8[:])
        nc.gpsimd.collective_compute(
            "AllGather", Alu.bypass, replica_groups=GROUPS,
            ins=[contrib[:]], outs=[gathered[:]],
        )
        for c in range(N_CORES):
            nc.sync.dma_start(out=wT[:, :, ts(c, WS_ROWS)], in_=gathered[c])

        # ---- main token loop: tiles of 128 tokens -----------------------
        for m in range(MS):
            x_t = xin.tile([P, D_IN], F32, tag="x")
            nc.sync.dma_start(out=x_t[:], in_=x_v[m])

            absmax = scl.tile([P, 1], F32, tag="absmax")
            m1t = scl.tile([P, 1], F32, tag="m1")
            r = scl.tile([P, 1], F32, tag="r")
            inv127 = scl.tile([P, 1], F32, tag="inv127")
            c_vec = scl.tile([P, 1], F32, tag="c_vec")

            nc.vector.tensor_reduce(
                absmax[:], x_t[:], axis=AX.X, op=Alu.max, apply_absolute_value=True
            )
            nc.vector.tensor_scalar(m1t[:], absmax[:], EPS, None, op0=Alu.max)
            nc.vector.reciprocal(r[:], m1t[:])
            nc.scalar.mul(inv127[:], r[:], 127.0)
            nc.scalar.mul(c_vec[:], m1t[:], alpha127[:])

            # u = q + M1 (f32, exact int + magic), q = u - M1 as bf16 ints
            u_t = xu.tile([P, D_IN], F32, tag="u")
            nc.scalar.activation(u_t[:], x_t[:], Copy, bias=M1, scale=inv127[:])
            q_t = xq.tile([P, D_IN], BF16, tag="q")
            nc.vector.tensor_scalar(q_t[:], u_t[:], M1, None, op0=Alu.subtract)

            # transpose to [i, t] layout (ACT HWDGE ring, 2-byte xbar)
            xT_t = xt.tile([P, KC, P], BF16, tag="xT")
            nc.scalar.dma_start_transpose(xT_t[:], q_t[:])

            # split q = hi16 + lo (both fp8-exact) on the transposed side:
            #   t2 = q + M2 (rounds q to the 16-grid), hi16 = t2 - M2,
            #   lo = q - hi16 in [-8,8].
            t2 = xu.tile([P, KC, P], F32, tag="t2")
            nc.scalar.activation(t2[:], xT_t[:], Copy, bias=M2)
            xE = xe.tile([P, 2 * KC, P], FP8, tag="xE")
            nc.vector.tensor_scalar(xE[:, 0:KC, :], t2[:], M2, None,
                                    op0=Alu.subtract)
            s2 = xq.tile([P, KC, P], BF16, tag="s2")
            nc.gpsimd.tensor_scalar(s2[:], t2[:], M2, None, op0=Alu.subtract)
            nc.gpsimd.tensor_tensor(xE[:, KC:2 * KC, :], xT_t[:], s2[:],
                                    op=Alu.subtract)

            # 16 DoubleRow matmuls per n-chunk: 8 hi16 pairs + 8 lo pairs,
            # each pair of k-subtiles shares the single-copy fp8 wT.
            ps = psum.tile([P, NT, NFREE], F32, tag="mm")
            norder = range(NT)
            for n in norder:
                for j in range(KC):
                    wk = (2 * j) % KC
                    nc.tensor.matmul(
                        ps[:, n, :],
                        xE[:, 2 * j:2 * j + 2, :],
                        wT[:, wk:wk + 2, ts(n, NFREE)],
                        start=(j == 0),
                        stop=(j == KC - 1),
                        perf_mode=DR,
                    )
            y_t = yout.tile([P, D_OUT], F32, tag="y")
            nc.scalar.activation(
                y_t[:], ps.rearrange("p a b -> p (a b)"), Copy, scale=c_vec[:]
            )
            nc.gpsimd.tensor_tensor(
                y_t[:], y_t[:], bias_bc[:], op=Alu.add,
            )
            nc.sync.dma_start(out=y_v[m], in_=y_t[:])

    nc.compile()
    return nc


_PROG_CACHE: dict[tuple, bass.Bass] = {}


def _get_prog(T: int, repeat: int = 1) -> bass.Bass:
    key = (T, repeat)
    if key not in _PROG_CACHE:
        _PROG_CACHE[key] = _build(T, repeat)
    return _PROG_CACHE[key]


def _make_in_maps(xf: np.ndarray, w: np.ndarray, b: np.ndarray, T: int):
    return [
        {
            "x": np.ascontiguousarray(xf[c * T : (c + 1) * T]),
            "ws": np.ascontiguousarray(w[c * WS_ROWS : (c + 1) * WS_ROWS]),
            "b": b,
        }
        for c in range(N_CORES)
    ]


def kernel(x: np.ndarray, weight: np.ndarray, bias: np.ndarray) -> np.ndarray:
    orig_shape = x.shape
    xf = np.ascontiguousarray(x.reshape(-1, D_IN).astype(np.float32, copy=False))
    n_tok = xf.shape[0]
    assert n_tok % N_CORES == 0
    T = n_tok // N_CORES
    w = np.ascontiguousarray(weight.astype(np.float32, copy=False))
    b = np.ascontiguousarray(bias.astype(np.float32, copy=False))

    nc = _get_prog(T)
    in_maps = _make_in_maps(xf, w, b, T)
    res = run_bass_kernel_spmd(nc, in_maps, core_ids=list(range(N_CORES)))
    y = np.concatenate([r["y"] for r in res.results], axis=0)
    return y.reshape(orig_shape[:-1] + (D_OUT,)).astype(np.float32)
